# revision 25
# baseline (speedup 1.0000x reference)
"""Trainium2 Bass kernel: ClusterlingLayer (VQ codebook Student-t soft assignment).

reference (ALPHA=1):
    dist[b,k] = max(||x_b||^2 + ||w_k||^2 - 2 x_b.w_k, 0)
    q = (1 + dist)^-1, row-normalized

Data-parallel over batch across 8 NeuronCores, full I/O on host.

Device does exactly the compute-heavy part, nothing else:
    PSUM[b,k] = sum_d xq[b,d] * wq[k,d]      (fp8e4 DoubleRow matmuls,
        xq = e4m3(x * SX/c_b), wq = e4m3(-2*GW*w), c_b = 1+||x_b||^2)
    out = bf16(PSUM)                          (Vector h0 / Scalar h1 copies)
and streams `out` back.  Host reconstructs the exact reference math in fp32
(c_b, v_k = ||w_k||^2 known exactly on host):
    1+dist = c_b + v_k + PSUM * c_b/(SX*GW);  q = normalize(1/(1+dist)).
Only fp8 operand quantization + bf16 transport remain as error sources
(~7e-4 max rel err vs the 2e-2 gate).

Schedule notes (from trace analysis):
  - DMA trigger->first-packet latency ~0.9us, ~213GB/s per queue, and
    ~0.5us semaphore propagation on every cross-engine dep.  Inputs are
    cut into 5 pieces ordered by first use over 3 queues so the first
    DoubleRow matmul fires as early as possible and never starves.
  - 32 DR matmuls (contraction 256, moving 512) at 215ns cadence
    (full 2x fp8 rate, LDWEIGHTS hidden behind MATMUL).
  - PSUM is 8 half-tiles [128,512] (one bank each); copies release a
    half as soon as its 2 matmuls are done.  Group A runs chunk-phase
    order (input-arrival gated), group B tile-major so copies/DMAs
    stream out during the matmuls.  qu pool holds all 8 tiles so no
    write-after-read gating on SBUF slots.
"""

from contextlib import ExitStack

import numpy as np
import ml_dtypes

import concourse.bacc as bacc
import concourse.bass as bass
import concourse.mybir as mybir
import concourse.tile as tile
from concourse.bass_utils import run_bass_kernel_spmd

N_CORES = 8
B, D, K = 8192, 512, 1024
BL = B // N_CORES  # 1024 batch rows per core
P = 128
NSUB = D // P  # 4 contraction subtiles of 128
NH = K // 512  # 2 k-halves (one PSUM bank each)
NB = BL // P  # 8 b-tiles per core
GRP = 4
HB = BL // 2  # rows per tile-group (A: 0..511, B: 512..1023)

SX = 512.0  # x pre-scale (before /c_b)
GW = 32.0  # w pre-scale
SCALE = SX * GW

N_WARMUP_MM = 26  # HAM activity-ramp primer: ends right as real matmuls start

_CACHE: dict = {}
LAST_RESULTS = None


def _build_nc() -> bass.Bass:
    nc = bacc.Bacc("TRN2", debug=False, target_bir_lowering=False)
    bf16 = mybir.dt.bfloat16
    fp32 = mybir.dt.float32
    fp8 = mybir.dt.float8e4
    DR = mybir.MatmulPerfMode.DoubleRow

    # input pieces, ordered by first use; p0 is a small lead piece so the
    # first matmuls gate ~0.5us earlier (per-DMA completion latency ~2us
    # dominates, so pieces are otherwise big and one-per-queue)
    p0_d = nc.dram_tensor("p0", [P, 2, 1024], fp8, kind="ExternalInput")  # x01 A | w01h0
    pa_d = nc.dram_tensor("pa", [P, 2, 512], fp8, kind="ExternalInput")  # w01h1
    pb_d = nc.dram_tensor("pb", [P, 2, 1536], fp8, kind="ExternalInput")  # w23 | x23 A
    pc_d = nc.dram_tensor("pc", [P, NSUB, HB], fp8, kind="ExternalInput")  # x colsB
    q_d = nc.dram_tensor("q", [NB, P, K], bf16, kind="ExternalOutput")

    with tile.TileContext(nc) as tc, ExitStack() as ctx:
        const = ctx.enter_context(tc.tile_pool(name="const", bufs=1))

        scratch = const.tile([P, P], bf16, tag="scr", name="scr_t")
        nc.gpsimd.memset(scratch[:], 0.25)

        p0 = const.tile([P, 2, 1024], fp8, tag="p0", name="p0_t")
        pa = const.tile([P, 2, 512], fp8, tag="pa", name="pa_t")
        pb = const.tile([P, 2, 1536], fp8, tag="pb", name="pb_t")
        pc = const.tile([P, NSUB, HB], fp8, tag="pc", name="pc_t")

        nc.sync.dma_start(p0[:], p0_d[:, :, :])
        nc.scalar.dma_start(pb[:], pb_d[:, :, :])
        nc.gpsimd.dma_start(pc[:], pc_d[:, :, :])
        nc.sync.dma_start(pa[:], pa_d[:, :, :])

        psum_pool = ctx.enter_context(tc.tile_pool(name="ps", bufs=8, space="PSUM"))
        qup = ctx.enter_context(tc.tile_pool(name="qu", bufs=NB))

        def lhsT(j, c):
            if j < GRP:  # group A
                if c == 0:
                    return p0[:, 0:2, j * P : (j + 1) * P]
                return pb[:, 0:2, 1024 + j * P : 1024 + (j + 1) * P]
            jb = j - GRP
            return pc[:, 2 * c : 2 * c + 2, jb * P : (jb + 1) * P]

        def rhs(c, h):
            if c == 0:
                return p0[:, 0:2, 512:1024] if h == 0 else pa[:, 0:2, 0:512]
            return pb[:, 0:2, h * 512 : (h + 1) * 512]

        pss = {}

        def mk_psum(j):
            pss[j] = {
                h: psum_pool.tile(
                    [P, 512], fp32, name="ps", tag=f"ps{j % GRP}{h}", bufs=1
                )
                for h in range(NH)
            }

        def mm(j, c, h):
            nc.tensor.matmul(
                pss[j][h][:, :],
                lhsT=lhsT(j, c),
                rhs=rhs(c, h),
                start=(c == 0),
                stop=(c == 1),
                perf_mode=DR,
                skip_group_check=True,
            )

        def epilogue(j):
            qu = qup.tile([P, K], bf16, name="qu")
            nc.scalar.copy(qu[:, 0:512], pss[j][0][:, :])
            nc.vector.tensor_scalar_mul(qu[:, 512:1024], pss[j][1][:, :], 1.0)
            eng = nc.sync if (j % 2 == 0 or j == NB - 1) else nc.gpsimd
            eng.dma_start(q_d[j], qu[:])

        # group A: c0 phase (gated on pa), then c1 tile-major (gated on pb)
        for j in range(GRP):
            mk_psum(j)
        for _ in range(N_WARMUP_MM):
            nc.tensor.matmul(
                pss[0][0][:, 0:P],
                lhsT=scratch[:, :],
                rhs=scratch[:, :],
                start=True,
                stop=True,
                skip_group_check=True,
            )
        for j in range(GRP):  # h0 first: gated only on the p0 lead piece
            mm(j, 0, 0)
        for j in range(GRP):
            mm(j, 0, 1)
        for j in range(GRP):  # c1 tile-major: each tile's PSUM completes early
            mm(j, 1, 0)
            mm(j, 1, 1)
            epilogue(j)
        # group B: tile-major throughout (all data long since landed)
        for j in range(GRP, NB):
            mk_psum(j)
            mm(j, 0, 0)
            mm(j, 0, 1)
            mm(j, 1, 0)
            mm(j, 1, 1)
            epilogue(j)
    nc.compile()
    return nc


def _prep_inputs(x: np.ndarray, weight: np.ndarray):
    """Host-side shard + scale + quantize + pack. Returns in_maps; stashes
    epilogue constants (c per row, v per code)."""
    e4m3 = ml_dtypes.float8_e4m3
    x = np.asarray(x, dtype=np.float32)
    w = np.asarray(weight, dtype=np.float32)

    c = 1.0 + np.einsum("bd,bd->b", x.astype(np.float64), x.astype(np.float64))
    v = np.einsum("kd,kd->k", w.astype(np.float64), w.astype(np.float64))

    xs = (x * (SX / c[:, None]).astype(np.float32)).astype(e4m3)  # [B, D]
    wq = (-2.0 * GW * w).astype(e4m3)  # [K, D]

    def pcs(src):  # src [cols, D] -> [P, nsub, cols]; v[p,s,i] = src[i, 128s+p]
        n = src.shape[1] // P
        return np.ascontiguousarray(src.T.reshape(n, P, -1).transpose(1, 0, 2))

    wt = pcs(wq)  # [P, 4, K]
    in_maps = []
    for i in range(N_CORES):
        xc = pcs(xs[i * BL : (i + 1) * BL])  # [P, 4, BL]
        p0 = np.concatenate([xc[:, 0:2, 0:HB], wt[:, 0:2, 0:512]], axis=2)
        pa = np.ascontiguousarray(wt[:, 0:2, 512:1024])
        pb = np.concatenate([wt[:, 2:4, :], xc[:, 2:4, 0:HB]], axis=2)
        pc = np.ascontiguousarray(xc[:, :, HB:BL])
        in_maps.append({"p0": p0, "pa": pa, "pb": pb, "pc": pc})
    _CACHE["epilogue"] = (c.astype(np.float32), v.astype(np.float32))
    return in_maps


def _postprocess(res) -> np.ndarray:
    """Exact reference math from the raw GEMM output."""
    c, v = _CACHE["epilogue"]
    qs = []
    for i in range(N_CORES):
        out = np.asarray(res.results[i]["q"])  # [NB, P, K] bf16
        ps = out.astype(np.float32).reshape(BL, K)
        cc = c[i * BL : (i + 1) * BL]
        t = cc[:, None] + v[None, :] + ps * (cc / SCALE)[:, None]
        np.maximum(t, 1.0, out=t)  # reference's relu(dist) clamp
        y = 1.0 / t
        qs.append(y / y.sum(axis=1, keepdims=True))
    return np.concatenate(qs, axis=0)


def kernel(x: np.ndarray, weight: np.ndarray) -> np.ndarray:
    global LAST_RESULTS
    if "nc" not in _CACHE:
        _CACHE["nc"] = _build_nc()
    nc = _CACHE["nc"]
    in_maps = _prep_inputs(x, weight)
    res = run_bass_kernel_spmd(nc, in_maps, list(range(N_CORES)))
    LAST_RESULTS = res
    return _postprocess(res)


if __name__ == "__main__":
    rng = np.random.default_rng(0)
    x = rng.standard_normal((B, D), dtype=np.float32)
    w = (rng.random((K, D), dtype=np.float32) - 0.5) * 0.12
    q = kernel(x, w)
    print("q shape", q.shape, "row sums", q.sum(1)[:4])
